# revision 28
# baseline (speedup 1.0000x reference)
"""Quantized Linear (8-bit act / 4-bit weight fake-quant) on 8 Trainium2 cores.

Math (per reference):
  xq = rne(x / s_x) * s_x          s_x = max(absmax(x)/127, 1e-8)
  wq = rne(w / s_w) * s_w          s_w = max(absmax(w)/7,   1e-8)
  bq = rne(b / s_b) * s_b          s_b = max(absmax(b)/127, 1e-8)
  out_pre = bq + xq @ wq.T
  out = rne(out_pre / s_o) * s_o   s_o = max(absmax(out_pre)/127, 1e-8)

Device strategy (2-way tokens x 4-way out_features, 8 cores):
  - Host packs per-core inputs k-major so the contraction dim lands on SBUF
    partitions with 8-32KB contiguous DMA lines: x -> [8 blk, 128, 8192]
    (blk-major, [kt,t] flat per partition), w -> [128, 32*1024] ([kt,j] flat).
    The PE does ONLY matmuls; no transposes anywhere.
  - Quantized integers Qx in [-127,127] / Qw in [-7,7] are exact in bf16 and
    accumulate exactly in fp32 PSUM; scales fold in afterwards:
    out_pre = (Qx@Qw)*(s_x*s_w) + bq.
  - Round-to-nearest-even via the fp32 magic constant (t + 1.5*2^23 then
    subtract). Quantization runs in [128,2048] chunks (ACT magic-add in
    place, DVE subtract to bf16) to amortize per-instruction overheads.
  - Global absmaxes via exclusive slices + one tiny AllReduce-max: each core
    reads 1/8 of x (its first two token blocks, host-rotated; bytes land in
    the opre buffer and are consumed from there) and 1/8 of w (a k-tile
    half; the host k-rotates BOTH x and w per core, so programs stay
    SPMD-identical and contraction order is irrelevant). DVE absmax reduces
    are issued in expected data-arrival order to avoid head-of-line blocks;
    W main loads issue from the (idle) PE queue so no ring stalls them.
  - Per 256-token block: 8 PSUM half-bank accumulators; block 0 runs one
    8-bank group (it chases the W DMA stream anyway), later blocks run two
    jt-groups of 4 so eviction overlaps the other group's matmul. Second
    AllReduce-max over out_pre, then requantize + store per half block.
"""

import sys

sys.path.insert(0, "/opt/trn_rl_repo")

import numpy as np

import concourse.bass as bass
import concourse.mybir as mybir
import concourse.tile as tile
from concourse import bacc, bass_isa

F32 = mybir.dt.float32
BF16 = mybir.dt.bfloat16
AF = mybir.ActivationFunctionType
ALU = mybir.AluOpType
AX = mybir.AxisListType

MAGIC = 12582912.0  # 1.5 * 2**23: fp32 add rounds to nearest-even integer
EPS = 1e-8
INV_QA = float(np.float32(1.0) / np.float32(127.0))
INV_QW = float(np.float32(1.0) / np.float32(7.0))

P = 128
RT, RJ = 2, 4  # token groups x out-feature groups


def build(n_cores=8, T=4096, K=4096, J=4096, TB=256):
    TA = T // RT  # 2048 tokens per core
    JB = J // RJ  # 1024 out features per core
    n_kt = K // P  # 32 k-tiles
    n_tb = TA // TB  # 8 token blocks
    n_jt = JB // P  # 8 j-tiles
    BLK = n_kt * TB  # 8192 floats per partition per x block
    SLOT = n_jt * TB  # 2048 floats per opre block slot
    WPT = JB  # w floats per partition per k-tile (1024)
    CH = 2048  # elementwise chunk size
    NWC = n_kt * WPT // CH  # 16 w chunks
    NXC = BLK // CH  # 4 x chunks per block

    nc = bacc.Bacc(
        "TRN2", target_bir_lowering=False, debug=False, num_devices=n_cores
    )

    x_d = nc.dram_tensor("x_p", [n_tb, P, BLK], F32, kind="ExternalInput")
    w_d = nc.dram_tensor("w_p", [P, n_kt * WPT], F32, kind="ExternalInput")
    b_d = nc.dram_tensor("b_full", [P, J // P], F32, kind="ExternalInput")
    bs_d = nc.dram_tensor("b_shard", [1, JB], F32, kind="ExternalInput")
    o_d = nc.dram_tensor("out_s", [TA, JB], F32, kind="ExternalOutput")
    cc0_in = nc.dram_tensor("cc0_in", [1, 1], F32)
    cc0_out = nc.dram_tensor("cc0_out", [1, 1], F32)
    cc1_in = nc.dram_tensor("cc1_in", [1, 2], F32)
    cc1_out = nc.dram_tensor("cc1_out", [1, 2], F32)
    cc2_in = nc.dram_tensor("cc2_in", [1, 1], F32)
    cc2_out = nc.dram_tensor("cc2_out", [1, 1], F32)
    groups = [list(range(n_cores))]

    with tile.TileContext(nc) as tc:
        with (
            tc.tile_pool(name="const", bufs=1) as const,
            tc.tile_pool(name="scal", bufs=1) as scal,
            tc.tile_pool(name="qwp", bufs=1) as qwp,
            tc.tile_pool(name="qxp", bufs=2) as qxp,
            tc.tile_pool(name="big", bufs=1) as big,
            tc.tile_pool(name="stage", bufs=3) as stage,
            tc.tile_pool(name="typo", bufs=3) as typo,
            tc.tile_pool(name="mmps", bufs=8, space="PSUM") as mmps,
        ):
            bfull = scal.tile([P, J // P], F32)
            nc.gpsimd.dma_start(bfull[:], b_d[:, :])
            bsr = scal.tile([1, JB], F32)
            nc.gpsimd.dma_start(bsr[:], bs_d[:, :])
            # Warm-up collective: absorbs the one-time CC-ring setup (~10us
            # trigger delay) and core alignment while the exclusive-slice
            # DMAs stream, so the real absmax AllReduce starts instantly.
            nc.gpsimd.collective_compute(
                "AllReduce", ALU.max, replica_groups=[[c] for c in range(n_cores)],
                ins=[cc0_in[:]], outs=[cc0_out[:]],
            )
            magic_t = const.tile([P, 1], F32)
            nc.vector.memset(magic_t[:], MAGIC)

            # ---------------- Phase 0: exclusive-slice absmaxes -------------
            nax = 2 * (BLK // 2048)  # 8 x reduce columns
            naw = (n_kt // 2) * WPT // CH  # 8 w reduce columns
            am = scal.tile([P, nax + naw + 1], F32)

            opre = big.tile([P, n_tb * SLOT], F32)
            # issue all exclusive-slice DMAs first (x in 2MiB halves so the
            # absmax reduces can start as early as possible) ...
            for i in range(2):
                for h in range(2):
                    nc.sync.dma_start(
                        opre[:, i * BLK + h * 4096 : i * BLK + (h + 1) * 4096],
                        x_d[i, :, h * 4096 : (h + 1) * 4096],
                    )
            wex = []
            for e in range(naw):
                t = stage.tile([P, CH], F32, tag="st", name=f"wex_{e}")
                nc.scalar.dma_start(t[:], w_d[:, e * CH : (e + 1) * CH])
                wex.append(t)
            # ... then DVE reduces in expected arrival order (w pieces land
            # every ~3.5us; x block i completes at ~30/45us).
            def wred(e):
                nc.vector.tensor_reduce(
                    am[:, nax + e : nax + e + 1], wex[e][:],
                    axis=AX.X, op=ALU.max, apply_absolute_value=True,
                )

            def xred(i, h):
                nc.vector.tensor_reduce(
                    am[:, i * 4 + h : i * 4 + h + 1],
                    opre[:, i * BLK + h * CH : i * BLK + (h + 1) * CH],
                    axis=AX.X, op=ALU.max, apply_absolute_value=True,
                )

            # interleaved by expected arrival (x and w stream concurrently
            # on separate queues at roughly equal fabric share)
            wred(0)
            xred(0, 0)
            wred(1)
            xred(0, 1)
            wred(2)
            xred(0, 2)
            wred(3)
            xred(0, 3)
            wred(4)
            xred(1, 0)
            wred(5)
            xred(1, 1)
            wred(6)
            xred(1, 2)
            wred(7)
            xred(1, 3)
            nc.vector.tensor_reduce(
                am[:, nax + naw :], bfull[:], axis=AX.X, op=ALU.max,
                apply_absolute_value=True,
            )

            m2 = scal.tile([P, 2], F32)
            nc.vector.tensor_reduce(m2[:, 0:1], am[:, :nax], axis=AX.X, op=ALU.max)
            nc.vector.tensor_reduce(
                m2[:, 1:2], am[:, nax : nax + naw], axis=AX.X, op=ALU.max
            )
            g2 = scal.tile([P, 2], F32)
            nc.gpsimd.partition_all_reduce(
                g2[:], m2[:], channels=P, reduce_op=bass_isa.ReduceOp.max
            )
            nc.sync.dma_start(cc1_in[:], g2[:1, :])
            # First 4 W main chunks fit in free stage buffers: stream them
            # during the collective wait, when the DMA fabric is idle (after
            # the exclusive reads that gate the collective are done).
            wf = []
            for e in range(3):
                t = stage.tile([P, CH], F32, tag="st", name=f"wf_{e}")
                nc.sync.dma_start(t[:], w_d[:, e * CH : (e + 1) * CH])
                wf.append(t)
            nc.gpsimd.collective_compute(
                "AllReduce", ALU.max, replica_groups=groups,
                ins=[cc1_in[:]], outs=[cc1_out[:]],
            )
            gx = scal.tile([P, 2], F32)
            nc.sync.dma_start(gx[:1, :], cc1_out[:])
            # Stream the remaining W chunks as soon as the collective is done
            # (their stage buffers only free up once quantization consumes
            # the early chunks, so these must not block any earlier ring op).
            for e in range(3, NWC):
                t = stage.tile([P, CH], F32, tag="st", name=f"wf_{e}")
                nc.sync.dma_start(t[:], w_d[:, e * CH : (e + 1) * CH])
                wf.append(t)
            bc2 = scal.tile([P, 2], F32)
            nc.gpsimd.partition_broadcast(bc2[:], gx[:1, :], channels=P)

            s_x = scal.tile([P, 1], F32)
            s_w = scal.tile([P, 1], F32)
            nc.vector.tensor_scalar(s_x[:], bc2[:, 0:1], INV_QA, EPS, op0=ALU.mult, op1=ALU.max)
            inv_sx = scal.tile([P, 1], F32)
            nc.vector.reciprocal(inv_sx[:], s_x[:])
            nc.vector.tensor_scalar(s_w[:], bc2[:, 1:2], INV_QW, EPS, op0=ALU.mult, op1=ALU.max)
            inv_sw = scal.tile([P, 1], F32)
            nc.vector.reciprocal(inv_sw[:], s_w[:])

            # ---------------- Quantization helpers --------------------------
            qwT = qwp.tile([P, n_kt * WPT], BF16)
            qxb = {}

            def quant_chunk(p, q, qx, skip=0):
                if p < 2:
                    reg = opre[:, p * BLK + q * CH + skip : p * BLK + (q + 1) * CH]
                else:
                    reg = stage.tile([P, CH], F32, tag="st", name=f"xs_{p}_{q}")
                    nc.sync.dma_start(reg, x_d[p, :, q * CH : (q + 1) * CH])
                    reg = reg[:, skip:]
                nc.scalar.activation(
                    reg, reg, AF.Identity, bias=magic_t[:], scale=inv_sx[:]
                )
                nc.vector.tensor_scalar(
                    qx[:, q * CH + skip : (q + 1) * CH], reg, -MAGIC, None, op0=ALU.add
                )

            def quant_block(p):
                qx = qxp.tile([P, BLK], BF16, tag="qx", name=f"qx_{p}")
                for q in range(NXC):
                    quant_chunk(p, q, qx)
                return qx

            def wquant_chunk(e, skip=0):
                nc.scalar.activation(
                    wf[e][:, skip:], wf[e][:, skip:], AF.Identity,
                    bias=magic_t[:], scale=inv_sw[:],
                )
                nc.vector.tensor_scalar(
                    qwT[:, e * CH + skip : (e + 1) * CH], wf[e][:, skip:],
                    -MAGIC, None, op0=ALU.add,
                )

            # Interleave: block 0+1 x-chunks woven between w chunks so the
            # first matmuls (and the first eviction) are never queue-blocked.
            qxb[0] = qxp.tile([P, BLK], BF16, tag="qx", name="qx_0")
            qxb[1] = qxp.tile([P, BLK], BF16, tag="qx", name="qx_1")
            # micro-chunks first: just k-tile 0 of x and w, so the first
            # matmul issues ~4us after the scales land
            reg0 = opre[:, 0:TB]
            nc.scalar.activation(reg0, reg0, AF.Identity, bias=magic_t[:], scale=inv_sx[:])
            nc.vector.tensor_scalar(qxb[0][:, 0:TB], reg0, -MAGIC, None, op0=ALU.add)
            nc.scalar.activation(
                wf[0][:, 0:WPT], wf[0][:, 0:WPT], AF.Identity,
                bias=magic_t[:], scale=inv_sw[:],
            )
            nc.vector.tensor_scalar(
                qwT[:, 0:WPT], wf[0][:, 0:WPT], -MAGIC, None, op0=ALU.add
            )
            for e in range(NWC):
                if e < NXC:
                    quant_chunk(0, e, qxb[0], skip=TB if e == 0 else 0)
                wquant_chunk(e, skip=WPT if e == 0 else 0)
                if NWC - e <= NXC:
                    quant_chunk(1, e - (NWC - NXC), qxb[1])

            s_xw = scal.tile([P, 1], F32)
            nc.vector.tensor_tensor(out=s_xw[:], in0=s_x[:], in1=s_w[:], op=ALU.mult)
            s_b = scal.tile([P, 1], F32)
            bmax = scal.tile([P, 1], F32)
            nc.gpsimd.partition_all_reduce(
                bmax[:], am[:, nax + naw :], channels=P, reduce_op=bass_isa.ReduceOp.max
            )
            nc.vector.tensor_scalar(s_b[:], bmax[:], INV_QA, EPS, op0=ALU.mult, op1=ALU.max)
            inv_sb = scal.tile([P, 1], F32)
            nc.vector.reciprocal(inv_sb[:], s_b[:])
            nc.scalar.activation(
                bsr[:], bsr[:], AF.Identity, bias=magic_t[:1, :], scale=inv_sb[:1, :]
            )
            nc.vector.tensor_scalar(
                bsr[:], bsr[:], -MAGIC, s_b[:1, :], op0=ALU.add, op1=ALU.mult
            )
            bq_b = scal.tile([P, JB], F32)
            nc.gpsimd.partition_broadcast(bq_b[:], bsr[:1, :], channels=P)

            # ---------------- Main: matmul + evict --------------------------
            JH = WPT // 2  # 512 moving j columns per matmul

            omax = scal.tile([P, n_tb * 2], F32)
            nc.vector.memset(omax[:], 0.0)

            def evict(p, th, jh, ps):
                oc = opre[:, p * SLOT + th * WPT + jh * JH : p * SLOT + th * WPT + (jh + 1) * JH]
                nc.vector.scalar_tensor_tensor(
                    oc, ps[:], s_xw[:], bq_b[:, jh * JH : (jh + 1) * JH],
                    op0=ALU.mult, op1=ALU.add,
                )
                if jh == 1:
                    nc.vector.tensor_reduce(
                        omax[:, p * 2 + th : p * 2 + th + 1],
                        opre[:, p * SLOT + th * WPT : p * SLOT + (th + 1) * WPT],
                        axis=AX.X, op=ALU.max, apply_absolute_value=True,
                    )

            for p in range(n_tb):
                if p + 2 < n_tb:
                    qxb[p + 2] = quant_block(p + 2)
                qx = qxb.pop(p)
                ps = [
                    mmps.tile([P, JH], F32, tag="mm", name=f"ps_{p}_{i}")
                    for i in range(4)
                ]
                if p == 0:
                    # block 0 chases the W DMA stream: consume each k-tile
                    # once (all 4 accumulators) so the pace matches arrivals
                    for kt in range(n_kt):
                        for th in range(2):
                            for jh in range(2):
                                nc.tensor.matmul(
                                    ps[th * 2 + jh][:],
                                    lhsT=qx[:, kt * TB + th * P : kt * TB + (th + 1) * P],
                                    rhs=qwT[:, kt * WPT + jh * JH : kt * WPT + (jh + 1) * JH],
                                    start=(kt == 0),
                                    stop=(kt == n_kt - 1),
                                )
                    for th in range(2):
                        for jh in range(2):
                            evict(p, th, jh, ps[th * 2 + jh])
                else:
                    # two th-groups of 2 banks: group 0's eviction overlaps
                    # group 1's matmuls
                    for th in range(2):
                        for kt in range(n_kt):
                            for jh in range(2):
                                nc.tensor.matmul(
                                    ps[th * 2 + jh][:],
                                    lhsT=qx[:, kt * TB + th * P : kt * TB + (th + 1) * P],
                                    rhs=qwT[:, kt * WPT + jh * JH : kt * WPT + (jh + 1) * JH],
                                    start=(kt == 0),
                                    stop=(kt == n_kt - 1),
                                )
                        for jh in range(2):
                            evict(p, th, jh, ps[th * 2 + jh])


            # ---------------- Tail: global out absmax -> requantize ---------
            om1 = scal.tile([P, 1], F32)
            nc.vector.tensor_reduce(om1[:], omax[:], axis=AX.X, op=ALU.max)
            omr = scal.tile([P, 1], F32)
            nc.gpsimd.partition_all_reduce(
                omr[:], om1[:], channels=P, reduce_op=bass_isa.ReduceOp.max
            )
            nc.sync.dma_start(cc2_in[:], omr[:1, :])
            nc.gpsimd.collective_compute(
                "AllReduce", ALU.max, replica_groups=groups,
                ins=[cc2_in[:]], outs=[cc2_out[:]],
            )
            go = scal.tile([P, 1], F32)
            nc.sync.dma_start(go[:1, :], cc2_out[:])
            bco = scal.tile([P, 1], F32)
            nc.gpsimd.partition_broadcast(bco[:], go[:1, :], channels=P)
            s_o = scal.tile([P, 1], F32)
            nc.vector.tensor_scalar(s_o[:], bco[:], INV_QA, EPS, op0=ALU.mult, op1=ALU.max)
            inv_so = scal.tile([P, 1], F32)
            nc.vector.reciprocal(inv_so[:], s_o[:])

            for p in range(n_tb):
                for th in range(2):
                    src = opre[:, p * SLOT + th * WPT : p * SLOT + (th + 1) * WPT]
                    nc.scalar.activation(
                        src, src, AF.Identity, bias=magic_t[:], scale=inv_so[:]
                    )
                    res = typo.tile([P, WPT], F32, tag="ores", name=f"res_{p}_{th}")
                    nc.vector.tensor_scalar(
                        res[:], src, -MAGIC, s_o[:], op0=ALU.add, op1=ALU.mult
                    )
                    nc.sync.dma_start(
                        o_d[p * TB + th * P : p * TB + (th + 1) * P, :], res[:]
                    )

    nc.compile()
    return nc


def _pack_x(xa, a, bb, n_tb=8, TB=256, n_kt=32):
    # xa: [TA, K] token-slice for group a -> [n_tb, 128, n_kt*TB] packed,
    # k-rotated by a (matching w) and token-block-rotated by bb (excl-first).
    t = xa.reshape(n_tb, TB, n_kt, P).transpose(0, 3, 2, 1)  # [tb, p, kt, t]
    t = np.roll(t, -a * (n_kt // 2), axis=2)
    t = np.roll(t, -2 * bb, axis=0)
    return np.ascontiguousarray(t).reshape(n_tb, P, n_kt * TB)


def _pack_w(wb, a, n_kt=32):
    # wb: [JB, K] out-feature slice -> [128, n_kt*JB] packed, k-rotated by a.
    JB = wb.shape[0]
    t = wb.T.reshape(n_kt, P, JB).transpose(1, 0, 2)  # [p, kt, j]
    t = np.roll(t, -a * (n_kt // 2), axis=1)
    return np.ascontiguousarray(t).reshape(P, n_kt * JB)


def _run(nc, inputs, n_cores, T, K, J, trace=False):
    from concourse.bass_utils import run_bass_kernel_spmd

    TA, JB, TB = T // RT, J // RJ, 256
    n_tb = TA // TB
    x = np.ascontiguousarray(inputs["x"], dtype=np.float32)
    w = np.ascontiguousarray(inputs["weight"], dtype=np.float32)
    b = np.ascontiguousarray(inputs["b"], dtype=np.float32)
    in_maps = []
    for c in range(n_cores):
        a, bb = divmod(c, RJ)
        in_maps.append(
            {
                "x_p": _pack_x(x[a * TA : (a + 1) * TA], a, bb, n_tb, TB, K // P),
                "w_p": _pack_w(w[bb * JB : (bb + 1) * JB], a, K // P),
                "b_full": np.ascontiguousarray(b.reshape(P, J // P)),
                "b_shard": np.ascontiguousarray(b[bb * JB : (bb + 1) * JB].reshape(1, JB)),
            }
        )
    res = run_bass_kernel_spmd(nc, in_maps, core_ids=list(range(n_cores)), trace=trace)
    out = np.empty((T, J), dtype=np.float32)
    for c in range(n_cores):
        a, bb = divmod(c, RJ)
        ot = res.results[c]["out_s"]  # [TA, JB], token blocks rotated by bb
        ot = ot.reshape(n_tb, TB, JB)
        ot = np.roll(ot, 2 * bb, axis=0).reshape(TA, JB)
        out[a * TA : (a + 1) * TA, bb * JB : (bb + 1) * JB] = ot
    return out, res


_NC_CACHE = {}


def kernel(**inputs) -> np.ndarray:
    n_cores, T, K, J = 8, 4096, 4096, 4096
    key = (n_cores, T, K, J)
    if key not in _NC_CACHE:
        _NC_CACHE[key] = build(n_cores, T, K, J)
    out, _ = _run(_NC_CACHE[key], inputs, n_cores, T, K, J)
    return out
